# revision 6
# baseline (speedup 1.0000x reference)
"""Trainium2 Bass kernel for nn_AttentionLayer_78632261255284 (sparse_attention).

Strategy (8-way row sharding, fully transpose-free):
  Each core owns a slab of 512 query rows. The reachability-factor matrix
  slab is computed *transposed* ([4096 keys, 512 queries]) via the chain
  D_{k+1} = B^T @ D_k with lhsT = B tiles, which is exactly the layout the
  transposed attention scores need. Softmax uses the identity
  softmax(qk + log f) = f*exp(qk) / sum(f*exp(qk)) -- no log, no max
  subtraction (scores are bounded). The softmax denominator comes from an
  appended ones-column in V; the division is applied per head via a rank-1
  (ones x recip-row) PE broadcast. The output projection consumes the
  transposed per-head outputs directly as lhsT. Host adds bo at the end.

Numerics: fp16 operand storage everywhere (validated: L2 rel err ~5e-4),
fp32 PSUM accumulation. Factors are globally scaled by 2^-9 (cancels in
softmax normalization exactly); D3 is stored scaled by 1/4 to stay in fp16
range. All scale constants are powers of two (exact).
"""

import numpy as np

import concourse.bass as bass
import concourse.mybir as mybir
import concourse.tile as tile
from concourse import bacc
from concourse.bass_utils import run_bass_kernel_spmd

P = 128
N = 4096  # nodes (+virtual)
NB = N // P  # 32 node blocks
EMB = 512
ET = EMB // P  # 4 embed blocks
HEADS = 8
HD = 64
SLAB = 512  # rows per core
NCORES = 8
CHUNK = 8  # kb-blocks per attention chunk (sexp buffer granularity)

dt = mybir.dt
AF = mybir.ActivationFunctionType
ALU = mybir.AluOpType

# factor-scale constants (powers of two; global 2^-9 scale cancels in softmax)
FSCALE = 1.0 / 512.0
C2 = 0.5 * FSCALE
C3 = 0.25 * FSCALE
C4 = 0.125 * FSCALE
D3_STORE = 0.25  # D3 stored as D3/4 (fp16 range); level-3 psum is D4/4

_NC_CACHE = {}
LAST_RESULT = None


def _install_ntff_shim():
    """Provide antenv.axon_hooks if the image lacks it, so trace=True under
    axon works (profiling via ctypes into libaxon_pjrt.so). No-op if the
    real module exists or the .so lacks the symbols."""
    try:
        from antenv.axon_hooks import get_axon_ntff_profile_hook  # noqa: F401
        return
    except ImportError:
        pass
    import contextlib
    import ctypes
    import sys
    import types

    so_path = "/opt/axon/libaxon_pjrt.so"
    hook = None
    try:
        lib = ctypes.CDLL(so_path)
        if hasattr(lib, "axon_start_nrt_profile"):
            lib.axon_start_nrt_profile.argtypes = [
                ctypes.POINTER(ctypes.c_int64),
                ctypes.c_size_t,
            ]
            lib.axon_start_nrt_profile.restype = ctypes.c_int64
            lib.axon_stop_nrt_profile.argtypes = [ctypes.c_char_p]
            lib.axon_stop_nrt_profile.restype = ctypes.c_int64

            @contextlib.contextmanager
            def _hook(output_dir, device_ids):
                import jax

                jax.devices()
                if device_ids:
                    ids = (ctypes.c_int64 * len(device_ids))(*device_ids)
                    rc = lib.axon_start_nrt_profile(ids, len(device_ids))
                else:
                    rc = lib.axon_start_nrt_profile(None, 0)
                if rc != 0:
                    raise RuntimeError(f"axon_start_nrt_profile rc={rc}")
                try:
                    yield
                finally:
                    n = lib.axon_stop_nrt_profile(str(output_dir).encode())
                    if n < 0:
                        raise RuntimeError(f"axon_stop_nrt_profile rc={n}")

            hook = _hook
    except OSError:
        pass

    mod = types.ModuleType("antenv.axon_hooks")
    mod.get_axon_ntff_profile_hook = lambda: hook
    mod.set_axon_ntff_profile_hook = lambda h: None
    sys.modules["antenv.axon_hooks"] = mod


_install_ntff_shim()


def build_bass():
    nc = bacc.Bacc("TRN2", target_bir_lowering=False, debug=False, num_devices=NCORES)

    bt = nc.dram_tensor("bt", [NB, P, NB, P], dt.float16, kind="ExternalInput")
    d1 = nc.dram_tensor("d1", [N, SLAB], dt.float16, kind="ExternalInput")
    xt = nc.dram_tensor("xt", [EMB, N], dt.float16, kind="ExternalInput")
    xtr = nc.dram_tensor("xtr", [EMB, SLAB], dt.float16, kind="ExternalInput")
    wq = nc.dram_tensor("wq", [EMB, EMB], dt.float16, kind="ExternalInput")
    wk = nc.dram_tensor("wk", [EMB, EMB], dt.float16, kind="ExternalInput")
    wv = nc.dram_tensor("wv", [EMB, EMB], dt.float16, kind="ExternalInput")
    wo = nc.dram_tensor("wo", [EMB, EMB], dt.float16, kind="ExternalInput")
    bq = nc.dram_tensor("bq", [EMB], dt.float32, kind="ExternalInput")
    bk = nc.dram_tensor("bk", [EMB], dt.float32, kind="ExternalInput")
    bv = nc.dram_tensor("bv", [EMB], dt.float32, kind="ExternalInput")
    out = nc.dram_tensor("out", [SLAB, EMB], dt.float32, kind="ExternalOutput")

    with tile.TileContext(nc) as tc:
        with (
            tc.tile_pool(name="persist", bufs=1) as persist,
            tc.tile_pool(name="psA", bufs=3, space="PSUM") as psA,
            tc.tile_pool(name="psB", bufs=2, space="PSUM") as psB,
            tc.tile_pool(name="psR", bufs=1, space="PSUM") as psR,
        ):
            # ---------------- persistent tiles ----------------
            F = persist.tile([P, NB, SLAB], dt.float16, tag="F")
            qT = persist.tile([P, ET, SLAB], dt.float16, tag="qT")
            out_allT = persist.tile([P, ET, SLAB], dt.float16, tag="out_allT")
            wo_sb = persist.tile([P, ET, EMB], dt.float16, tag="wo_sb")
            bv_sb = persist.tile([P, ET], dt.float32, tag="bv_sb")
            ones64 = persist.tile([1, HD], dt.float16, tag="ones64")

            nc.sync.dma_start(wo_sb[:], wo.rearrange("(t p) c -> p t c", p=P))
            nc.sync.dma_start(bv_sb[:], bv.rearrange("(t p) -> p t", p=P))
            nc.vector.memset(ones64[:], 1.0)

            # ---------------- phase M: mask chain ----------------
            with tc.tile_pool(name="dchain", bufs=1) as dchain, tc.tile_pool(
                name="btile", bufs=3
            ) as btile:
                D_a = dchain.tile([P, NB, SLAB], dt.float16, tag="D_a")
                D_b = dchain.tile([P, NB, SLAB], dt.float16, tag="D_b")
                nc.sync.dma_start(D_a[:], d1.rearrange("(kb p) q -> p kb q", p=P))

                for level in range(3):
                    src = D_a if level % 2 == 0 else D_b
                    dst = D_b if level % 2 == 0 else D_a
                    for m in range(NB):
                        bm = btile.tile([P, NB, P], dt.float16, tag="bm")
                        nc.sync.dma_start(bm[:], bt[m])
                        ps = psA.tile([P, SLAB], dt.float32, tag="acc")
                        for kb in range(NB):
                            nc.tensor.matmul(
                                ps[:],
                                bm[:, kb, :],
                                src[:, kb, :],
                                start=(kb == 0),
                                stop=(kb == NB - 1),
                            )
                        if level == 0:
                            nc.vector.tensor_scalar_mul(F[:, m, :], ps[:], C2)
                            nc.scalar.copy(dst[:, m, :], ps[:])
                        elif level == 1:
                            nc.vector.scalar_tensor_tensor(
                                out=F[:, m, :], in0=ps[:], scalar=C3, in1=F[:, m, :],
                                op0=ALU.mult, op1=ALU.max,
                            )
                            nc.scalar.mul(dst[:, m, :], ps[:], D3_STORE)
                        else:
                            nc.vector.scalar_tensor_tensor(
                                out=F[:, m, :], in0=ps[:], scalar=C4 * 4.0, in1=F[:, m, :],
                                op0=ALU.mult, op1=ALU.max,
                            )

            # ---------------- phase P: projections ----------------
            with tc.tile_pool(name="kv", bufs=1) as kv:
                kT = kv.tile([P, ET, N], dt.float16, tag="kT")
                v_sb = kv.tile([P, NB, HEADS, HD + 1], dt.float16, tag="v_sb")

                with tc.tile_pool(name="proj", bufs=1) as proj:
                    xt_sb = proj.tile([P, ET, N], dt.float16, tag="xt_sb")
                    xtr_sb = proj.tile([P, ET, SLAB], dt.float16, tag="xtr_sb")
                    wq_sb = proj.tile([P, ET, EMB], dt.float16, tag="wq_sb")
                    wk_sb = proj.tile([P, ET, EMB], dt.float16, tag="wk_sb")
                    wv_sb = proj.tile([P, ET, EMB], dt.float16, tag="wv_sb")
                    bq_sb = proj.tile([P, ET], dt.float32, tag="bq_sb")
                    bk_sb = proj.tile([P, ET], dt.float32, tag="bk_sb")

                    nc.sync.dma_start(xt_sb[:], xt.rearrange("(t p) n -> p t n", p=P))
                    nc.sync.dma_start(xtr_sb[:], xtr.rearrange("(t p) q -> p t q", p=P))
                    nc.sync.dma_start(wq_sb[:], wq.rearrange("(t p) c -> p t c", p=P))
                    nc.sync.dma_start(wk_sb[:], wk.rearrange("(t p) c -> p t c", p=P))
                    nc.sync.dma_start(wv_sb[:], wv.rearrange("(t p) c -> p t c", p=P))
                    nc.sync.dma_start(bq_sb[:], bq.rearrange("(t p) -> p t", p=P))
                    nc.sync.dma_start(bk_sb[:], bk.rearrange("(t p) -> p t", p=P))

                    # qT[hd, q] = (Wq' X_r^T) + bq'
                    for hb in range(ET):
                        ps = psA.tile([P, SLAB], dt.float32, tag="acc")
                        for t in range(ET):
                            nc.tensor.matmul(
                                ps[:],
                                wq_sb[:, t, hb * P : (hb + 1) * P],
                                xtr_sb[:, t, :],
                                start=(t == 0),
                                stop=(t == ET - 1),
                            )
                        nc.scalar.activation(
                            qT[:, hb, :], ps[:], AF.Identity, bias=bq_sb[:, hb : hb + 1]
                        )

                    # kT[hd, key] = (Wk X^T) + bk
                    for hb in range(ET):
                        for nck in range(N // SLAB):
                            ps = psA.tile([P, SLAB], dt.float32, tag="acc")
                            for t in range(ET):
                                nc.tensor.matmul(
                                    ps[:],
                                    wk_sb[:, t, hb * P : (hb + 1) * P],
                                    xt_sb[:, t, nck * SLAB : (nck + 1) * SLAB],
                                    start=(t == 0),
                                    stop=(t == ET - 1),
                                )
                            nc.scalar.activation(
                                kT[:, hb, nck * SLAB : (nck + 1) * SLAB],
                                ps[:],
                                AF.Identity,
                                bias=bk_sb[:, hb : hb + 1],
                            )

                    # V[node, hd] = X Wv   (bv added later per-partition on out'^T)
                    for nb in range(NB):
                        ps = psA.tile([P, SLAB], dt.float32, tag="acc")
                        for t in range(ET):
                            nc.tensor.matmul(
                                ps[:],
                                xt_sb[:, t, nb * P : (nb + 1) * P],
                                wv_sb[:, t, :],
                                start=(t == 0),
                                stop=(t == ET - 1),
                            )
                        nc.vector.tensor_copy(
                            v_sb[:, nb, :, 0:HD],
                            ps.rearrange("p (h d) -> p h d", h=HEADS),
                        )
                    nc.vector.memset(v_sb[:, :, :, HD : HD + 1], 1.0)

                # ---------------- phase A: attention ----------------
                with tc.tile_pool(name="attn", bufs=2) as attn, tc.tile_pool(
                    name="small", bufs=2
                ) as small:
                    for h in range(HEADS):
                        th = h // 2
                        po = (h % 2) * HD
                        po_tile = psB.tile([P, SLAB], dt.float32, tag="pout")
                        for ck in range(NB // CHUNK):
                            sexp = attn.tile([P, CHUNK, SLAB], dt.float16, tag="sexp")
                            for kc in range(CHUNK):
                                kb = ck * CHUNK + kc
                                ps = psA.tile([P, SLAB], dt.float32, tag="acc")
                                nc.tensor.matmul(
                                    ps[:],
                                    kT[po : po + HD, th, kb * P : (kb + 1) * P],
                                    qT[po : po + HD, th, :],
                                    start=True,
                                    stop=True,
                                )
                                nc.scalar.activation(sexp[:, kc, :], ps[:], AF.Exp)
                                nc.vector.tensor_tensor(
                                    out=sexp[:, kc, :], in0=sexp[:, kc, :],
                                    in1=F[:, kb, :], op=ALU.mult,
                                )
                            for kc in range(CHUNK):
                                kb = ck * CHUNK + kc
                                nc.tensor.matmul(
                                    po_tile[0 : HD + 1, :],
                                    v_sb[:, kb, h, :],
                                    sexp[:, kc, :],
                                    start=(kb == 0),
                                    stop=(kb == NB - 1),
                                )

                        # softmax denominator: row HD holds sum(f*exp)
                        row = small.tile([1, SLAB], dt.float32, tag="row")
                        nc.vector.tensor_copy(row[:], po_tile[HD : HD + 1, :])
                        nc.vector.reciprocal(row[:], row[:])
                        row16 = small.tile([1, SLAB], dt.float16, tag="row16")
                        nc.vector.tensor_copy(row16[:], row[:])
                        rps = psR.tile([HD, SLAB], dt.float32, tag="rps")
                        nc.tensor.matmul(rps[:], ones64[:], row16[:], start=True, stop=True)
                        r_sb = small.tile([HD, SLAB], dt.float32, tag="r_sb")
                        nc.scalar.copy(r_sb[:], rps[:])

                        otmp = small.tile([HD, SLAB], dt.float32, tag="otmp")
                        nc.vector.tensor_tensor(
                            out=otmp[:], in0=po_tile[0:HD, :], in1=r_sb[:], op=ALU.mult
                        )
                        nc.vector.tensor_scalar_add(
                            out_allT[po : po + HD, th, :], otmp[:],
                            bv_sb[po : po + HD, th : th + 1],
                        )

                    # ---------------- phase O: output projection ----------------
                    for qb in range(ET):
                        ps = psA.tile([P, SLAB], dt.float32, tag="acc")
                        for t in range(ET):
                            nc.tensor.matmul(
                                ps[:],
                                out_allT[:, t, qb * P : (qb + 1) * P],
                                wo_sb[:, t, :],
                                start=(t == 0),
                                stop=(t == ET - 1),
                            )
                        fin = small.tile([P, SLAB], dt.float32, tag="fin")
                        nc.vector.tensor_copy(fin[:], ps[:])
                        nc.sync.dma_start(out[qb * P : (qb + 1) * P, :], fin[:])

    nc.compile()
    return nc


def _prep_host(input_embeddings, edge_index, num_nodes, Wq, bq, Wk, bk, Wv, bv, Wo, bo):
    n = int(num_nodes) + 1
    assert n == N

    B = np.zeros((n, n), dtype=np.float32)
    idx = np.arange(n)
    B[idx, idx] = 1.0
    e0 = np.asarray(edge_index[0], dtype=np.int64)
    e1 = np.asarray(edge_index[1], dtype=np.int64)
    B[e0, e1] = 1.0
    B[: n - 1, n - 1] = 1.0
    B[n - 1, : n - 1] = 1.0

    B16 = B.astype(np.float16)
    # bt[m, p, kb, f] = B[kb*128+p, m*128+f]
    bt = np.ascontiguousarray(
        B16.reshape(NB, P, NB, P).transpose(2, 1, 0, 3)
    )

    X = np.asarray(input_embeddings, dtype=np.float32)
    xt = np.ascontiguousarray(X.T.astype(np.float16))

    wq_h = np.ascontiguousarray((np.asarray(Wq, np.float32) * 0.125).astype(np.float16))
    wk_h = np.ascontiguousarray(np.asarray(Wk, np.float32).astype(np.float16))
    wv_h = np.ascontiguousarray(np.asarray(Wv, np.float32).astype(np.float16))
    wo_h = np.ascontiguousarray(np.asarray(Wo, np.float32).astype(np.float16))
    bq_h = np.ascontiguousarray(np.asarray(bq, np.float32) * 0.125)
    bk_h = np.ascontiguousarray(np.asarray(bk, np.float32))
    bv_h = np.ascontiguousarray(np.asarray(bv, np.float32))

    in_maps = []
    for core in range(NCORES):
        r0 = core * SLAB
        d1 = np.ascontiguousarray(B16[r0 : r0 + SLAB, :].T)
        xtr = np.ascontiguousarray(xt[:, r0 : r0 + SLAB])
        in_maps.append(
            {
                "bt": bt,
                "d1": d1,
                "xt": xt,
                "xtr": xtr,
                "wq": wq_h,
                "wk": wk_h,
                "wv": wv_h,
                "wo": wo_h,
                "bq": bq_h,
                "bk": bk_h,
                "bv": bv_h,
            }
        )
    return in_maps


def kernel(**inputs) -> np.ndarray:
    if "nc" not in _NC_CACHE:
        _NC_CACHE["nc"] = build_bass()
    nc = _NC_CACHE["nc"]

    in_maps = _prep_host(**inputs)
    res = run_bass_kernel_spmd(nc, in_maps, core_ids=list(range(NCORES)))
    global LAST_RESULT
    LAST_RESULT = res
    bo = np.asarray(inputs["bo"], dtype=np.float32)
    slabs = [res.results[c]["out"] for c in range(NCORES)]
    return (np.concatenate(slabs, axis=0) + bo[None, :]).astype(np.float32)


if __name__ == "__main__":
    import reference

    inputs = {k: np.asarray(v) if not np.isscalar(v) else v for k, v in reference.setup_inputs().items()}
    got = kernel(**inputs)
    print("kernel output:", got.shape, got.dtype)


# revision 12
# speedup vs baseline: 1.1885x; 1.1885x over previous
"""Trainium2 Bass kernel for nn_AttentionLayer_78632261255284 (sparse_attention).

Strategy (8-way row sharding, fully transpose-free):
  Each core owns a slab of 512 query rows. The reachability-factor matrix
  slab is computed *transposed* ([4096 keys, 512 queries]) via the chain
  D_{k+1} = B^T @ D_k with lhsT = B tiles, which is exactly the layout the
  transposed attention scores need. Softmax uses the identity
  softmax(qk + log f) = f*exp(qk) / sum(f*exp(qk)) -- no log, no max
  subtraction (scores are bounded). The softmax denominator comes from an
  appended ones-column in V; the division is applied per head via a rank-1
  (ones x recip-row) PE broadcast. The output projection consumes the
  transposed per-head outputs directly as lhsT. Host adds bo at the end.

Numerics: fp16 operand storage everywhere (validated: L2 rel err ~5e-4),
fp32 PSUM accumulation. Factors are globally scaled by 2^-9 (cancels in
softmax normalization exactly); D3 is stored scaled by 1/4 to stay in fp16
range. All scale constants are powers of two (exact).
"""

import numpy as np

import concourse.bass as bass
import concourse.mybir as mybir
import concourse.tile as tile
from concourse import bacc
from concourse.bass_utils import run_bass_kernel_spmd

P = 128
N = 4096  # nodes (+virtual)
NB = N // P  # 32 node blocks
EMB = 512
ET = EMB // P  # 4 embed blocks
HEADS = 8
HD = 64
SLAB = 512  # rows per core
NCORES = 8
CHUNK = 32  # kb-blocks per attention chunk (full-head sexp, double-buffered)

dt = mybir.dt
AF = mybir.ActivationFunctionType
ALU = mybir.AluOpType

# factor-scale constants (powers of two; global 2^-9 scale cancels in softmax)
FSCALE = 1.0 / 512.0
C2 = 0.5 * FSCALE
C3 = 0.25 * FSCALE
C4 = 0.125 * FSCALE
D3_STORE = 0.25  # D3 stored as D3/4 (fp16 range); level-3 psum is D4/4

_NC_CACHE = {}
LAST_RESULT = None


def _install_ntff_shim():
    """Provide antenv.axon_hooks if the image lacks it, so trace=True under
    axon works (profiling via ctypes into libaxon_pjrt.so). No-op if the
    real module exists or the .so lacks the symbols."""
    try:
        from antenv.axon_hooks import get_axon_ntff_profile_hook  # noqa: F401
        return
    except ImportError:
        pass
    import contextlib
    import ctypes
    import sys
    import types

    so_path = "/opt/axon/libaxon_pjrt.so"
    hook = None
    try:
        lib = ctypes.CDLL(so_path)
        if hasattr(lib, "axon_start_nrt_profile"):
            lib.axon_start_nrt_profile.argtypes = [
                ctypes.POINTER(ctypes.c_int64),
                ctypes.c_size_t,
            ]
            lib.axon_start_nrt_profile.restype = ctypes.c_int64
            lib.axon_stop_nrt_profile.argtypes = [ctypes.c_char_p]
            lib.axon_stop_nrt_profile.restype = ctypes.c_int64

            @contextlib.contextmanager
            def _hook(output_dir, device_ids):
                import jax

                jax.devices()
                if device_ids:
                    ids = (ctypes.c_int64 * len(device_ids))(*device_ids)
                    rc = lib.axon_start_nrt_profile(ids, len(device_ids))
                else:
                    rc = lib.axon_start_nrt_profile(None, 0)
                if rc != 0:
                    raise RuntimeError(f"axon_start_nrt_profile rc={rc}")
                try:
                    yield
                finally:
                    n = lib.axon_stop_nrt_profile(str(output_dir).encode())
                    if n < 0:
                        raise RuntimeError(f"axon_stop_nrt_profile rc={n}")

            hook = _hook
    except OSError:
        pass

    mod = types.ModuleType("antenv.axon_hooks")
    mod.get_axon_ntff_profile_hook = lambda: hook
    mod.set_axon_ntff_profile_hook = lambda h: None
    sys.modules["antenv.axon_hooks"] = mod


_install_ntff_shim()


def build_bass():
    nc = bacc.Bacc("TRN2", target_bir_lowering=False, debug=False, num_devices=NCORES)

    bt = nc.dram_tensor("bt", [NB, P, NB, P], dt.float16, kind="ExternalInput")
    d1 = nc.dram_tensor("d1", [N, SLAB], dt.float16, kind="ExternalInput")
    xt = nc.dram_tensor("xt", [EMB, N], dt.float16, kind="ExternalInput")
    xtr = nc.dram_tensor("xtr", [EMB, SLAB], dt.float16, kind="ExternalInput")
    wq = nc.dram_tensor("wq", [EMB, EMB], dt.float16, kind="ExternalInput")
    wk = nc.dram_tensor("wk", [EMB, EMB], dt.float16, kind="ExternalInput")
    wv = nc.dram_tensor("wv", [EMB, EMB], dt.float16, kind="ExternalInput")
    wo = nc.dram_tensor("wo", [EMB, EMB], dt.float16, kind="ExternalInput")
    bq = nc.dram_tensor("bq", [EMB], dt.float32, kind="ExternalInput")
    bk = nc.dram_tensor("bk", [EMB], dt.float32, kind="ExternalInput")
    bv = nc.dram_tensor("bv", [EMB], dt.float32, kind="ExternalInput")
    out = nc.dram_tensor("out", [SLAB, EMB], dt.float32, kind="ExternalOutput")

    with tile.TileContext(nc) as tc:
        with (
            tc.tile_pool(name="persist", bufs=1) as persist,
            tc.tile_pool(name="psA", bufs=3, space="PSUM") as psA,
            tc.tile_pool(name="psB", bufs=2, space="PSUM") as psB,
            tc.tile_pool(name="psR", bufs=1, space="PSUM") as psR,
        ):
            # ---------------- persistent tiles ----------------
            F = persist.tile([P, NB, SLAB], dt.float16, tag="F")
            qT = persist.tile([P, ET, SLAB], dt.float16, tag="qT")
            out_allT = persist.tile([P, ET, SLAB], dt.float16, tag="out_allT")
            wo_sb = persist.tile([P, ET, EMB], dt.float16, tag="wo_sb")
            bv_sb = persist.tile([P, ET], dt.float32, tag="bv_sb")
            ones64 = persist.tile([1, HD], dt.float16, tag="ones64")

            nc.sync.dma_start(wo_sb[:], wo.rearrange("(t p) c -> p t c", p=P))
            nc.sync.dma_start(bv_sb[:], bv.rearrange("(t p) -> p t", p=P))
            nc.vector.memset(ones64[:], 1.0)

            # ---------------- phase M: mask chain ----------------
            with tc.tile_pool(name="dchain", bufs=1) as dchain, tc.tile_pool(
                name="btile", bufs=3
            ) as btile:
                D_a = dchain.tile([P, NB, SLAB], dt.float16, tag="D_a")
                D_b = dchain.tile([P, NB, SLAB], dt.float16, tag="D_b")
                nc.sync.dma_start(D_a[:], d1.rearrange("(kb p) q -> p kb q", p=P))

                for level in range(3):
                    src = D_a if level % 2 == 0 else D_b
                    dst = D_b if level % 2 == 0 else D_a
                    for m in range(NB):
                        bm = btile.tile([P, NB, P], dt.float16, tag="bm")
                        nc.sync.dma_start(bm[:], bt[m])
                        ps = psA.tile([P, SLAB], dt.float32, tag="acc")
                        for kb in range(NB):
                            nc.tensor.matmul(
                                ps[:],
                                bm[:, kb, :],
                                src[:, kb, :],
                                start=(kb == 0),
                                stop=(kb == NB - 1),
                            )
                        if level == 0:
                            nc.vector.tensor_scalar_mul(F[:, m, :], ps[:], C2)
                            nc.scalar.copy(dst[:, m, :], ps[:])
                        elif level == 1:
                            nc.vector.scalar_tensor_tensor(
                                out=F[:, m, :], in0=ps[:], scalar=C3, in1=F[:, m, :],
                                op0=ALU.mult, op1=ALU.max,
                            )
                            nc.scalar.mul(dst[:, m, :], ps[:], D3_STORE)
                        else:
                            nc.vector.scalar_tensor_tensor(
                                out=F[:, m, :], in0=ps[:], scalar=C4 * 4.0, in1=F[:, m, :],
                                op0=ALU.mult, op1=ALU.max,
                            )

            # ---------------- phase P: projections ----------------
            with tc.tile_pool(name="kv", bufs=1) as kv:
                kT = kv.tile([P, ET, N], dt.float16, tag="kT")
                v_sb = kv.tile([P, NB, HEADS, HD + 1], dt.float16, tag="v_sb")

                with tc.tile_pool(name="proj", bufs=1) as proj:
                    xt_sb = proj.tile([P, ET, N], dt.float16, tag="xt_sb")
                    xtr_sb = proj.tile([P, ET, SLAB], dt.float16, tag="xtr_sb")
                    wq_sb = proj.tile([P, ET, EMB], dt.float16, tag="wq_sb")
                    wk_sb = proj.tile([P, ET, EMB], dt.float16, tag="wk_sb")
                    wv_sb = proj.tile([P, ET, EMB], dt.float16, tag="wv_sb")
                    bq_sb = proj.tile([P, ET], dt.float32, tag="bq_sb")
                    bk_sb = proj.tile([P, ET], dt.float32, tag="bk_sb")

                    nc.sync.dma_start(xt_sb[:], xt.rearrange("(t p) n -> p t n", p=P))
                    nc.sync.dma_start(xtr_sb[:], xtr.rearrange("(t p) q -> p t q", p=P))
                    nc.sync.dma_start(wq_sb[:], wq.rearrange("(t p) c -> p t c", p=P))
                    nc.sync.dma_start(wk_sb[:], wk.rearrange("(t p) c -> p t c", p=P))
                    nc.sync.dma_start(wv_sb[:], wv.rearrange("(t p) c -> p t c", p=P))
                    nc.sync.dma_start(bq_sb[:], bq.rearrange("(t p) -> p t", p=P))
                    nc.sync.dma_start(bk_sb[:], bk.rearrange("(t p) -> p t", p=P))

                    # qT[hd, q] = (Wq' X_r^T) + bq'
                    for hb in range(ET):
                        ps = psA.tile([P, SLAB], dt.float32, tag="acc")
                        for t in range(ET):
                            nc.tensor.matmul(
                                ps[:],
                                wq_sb[:, t, hb * P : (hb + 1) * P],
                                xtr_sb[:, t, :],
                                start=(t == 0),
                                stop=(t == ET - 1),
                            )
                        nc.scalar.activation(
                            qT[:, hb, :], ps[:], AF.Identity, bias=bq_sb[:, hb : hb + 1]
                        )

                    # kT[hd, key] = (Wk X^T) + bk
                    for hb in range(ET):
                        for nck in range(N // SLAB):
                            ps = psA.tile([P, SLAB], dt.float32, tag="acc")
                            for t in range(ET):
                                nc.tensor.matmul(
                                    ps[:],
                                    wk_sb[:, t, hb * P : (hb + 1) * P],
                                    xt_sb[:, t, nck * SLAB : (nck + 1) * SLAB],
                                    start=(t == 0),
                                    stop=(t == ET - 1),
                                )
                            nc.scalar.activation(
                                kT[:, hb, nck * SLAB : (nck + 1) * SLAB],
                                ps[:],
                                AF.Identity,
                                bias=bk_sb[:, hb : hb + 1],
                            )

                    # V[node, hd] = X Wv   (bv added later per-partition on out'^T)
                    for nb in range(NB):
                        ps = psA.tile([P, SLAB], dt.float32, tag="acc")
                        for t in range(ET):
                            nc.tensor.matmul(
                                ps[:],
                                xt_sb[:, t, nb * P : (nb + 1) * P],
                                wv_sb[:, t, :],
                                start=(t == 0),
                                stop=(t == ET - 1),
                            )
                        nc.vector.tensor_copy(
                            v_sb[:, nb, :, 0:HD],
                            ps.rearrange("p (h d) -> p h d", h=HEADS),
                        )
                    nc.vector.memset(v_sb[:, :, :, HD : HD + 1], 1.0)

                # ---------------- phase A: attention ----------------
                with tc.tile_pool(name="attn", bufs=2) as attn, tc.tile_pool(
                    name="small", bufs=2
                ) as small:
                    for h in range(HEADS):
                        th = h // 2
                        po = (h % 2) * HD
                        po_tile = psB.tile([P, SLAB], dt.float32, tag="pout")
                        for ck in range(NB // CHUNK):
                            sexp = attn.tile([P, CHUNK, SLAB], dt.float16, tag="sexp")
                            for kc in range(CHUNK):
                                kb = ck * CHUNK + kc
                                ps = psA.tile([P, SLAB], dt.float32, tag="acc")
                                nc.tensor.matmul(
                                    ps[:],
                                    kT[po : po + HD, th, kb * P : (kb + 1) * P],
                                    qT[po : po + HD, th, :],
                                    start=True,
                                    stop=True,
                                )
                                nc.scalar.activation(sexp[:, kc, :], ps[:], AF.Exp)
                                nc.vector.tensor_tensor(
                                    out=sexp[:, kc, :], in0=sexp[:, kc, :],
                                    in1=F[:, kb, :], op=ALU.mult,
                                )
                            for kc in range(CHUNK):
                                kb = ck * CHUNK + kc
                                nc.tensor.matmul(
                                    po_tile[0 : HD + 1, :],
                                    v_sb[:, kb, h, :],
                                    sexp[:, kc, :],
                                    start=(kb == 0),
                                    stop=(kb == NB - 1),
                                )

                        # softmax denominator: row HD holds sum(f*exp)
                        row = small.tile([1, SLAB], dt.float32, tag="row")
                        nc.vector.tensor_copy(row[:], po_tile[HD : HD + 1, :])
                        nc.vector.reciprocal(row[:], row[:])
                        row16 = small.tile([1, SLAB], dt.float16, tag="row16")
                        nc.vector.tensor_copy(row16[:], row[:])
                        rps = psR.tile([HD, SLAB], dt.float32, tag="rps")
                        nc.tensor.matmul(rps[:], ones64[:], row16[:], start=True, stop=True)
                        r_sb = small.tile([HD, SLAB], dt.float32, tag="r_sb")
                        nc.scalar.copy(r_sb[:], rps[:])

                        otmp = small.tile([HD, SLAB], dt.float32, tag="otmp")
                        nc.vector.tensor_tensor(
                            out=otmp[:], in0=po_tile[0:HD, :], in1=r_sb[:], op=ALU.mult
                        )
                        nc.vector.tensor_scalar_add(
                            out_allT[po : po + HD, th, :], otmp[:],
                            bv_sb[po : po + HD, th : th + 1],
                        )

                    # ---------------- phase O: output projection ----------------
                    for qb in range(ET):
                        ps = psA.tile([P, SLAB], dt.float32, tag="acc")
                        for t in range(ET):
                            nc.tensor.matmul(
                                ps[:],
                                out_allT[:, t, qb * P : (qb + 1) * P],
                                wo_sb[:, t, :],
                                start=(t == 0),
                                stop=(t == ET - 1),
                            )
                        fin = small.tile([P, SLAB], dt.float32, tag="fin")
                        nc.vector.tensor_copy(fin[:], ps[:])
                        nc.sync.dma_start(out[qb * P : (qb + 1) * P, :], fin[:])

    nc.compile()
    return nc


def _prep_host(input_embeddings, edge_index, num_nodes, Wq, bq, Wk, bk, Wv, bv, Wo, bo):
    n = int(num_nodes) + 1
    assert n == N

    B = np.zeros((n, n), dtype=np.float32)
    idx = np.arange(n)
    B[idx, idx] = 1.0
    e0 = np.asarray(edge_index[0], dtype=np.int64)
    e1 = np.asarray(edge_index[1], dtype=np.int64)
    B[e0, e1] = 1.0
    B[: n - 1, n - 1] = 1.0
    B[n - 1, : n - 1] = 1.0

    B16 = B.astype(np.float16)
    # bt[m, p, kb, f] = B[kb*128+p, m*128+f]
    bt = np.ascontiguousarray(
        B16.reshape(NB, P, NB, P).transpose(2, 1, 0, 3)
    )

    X = np.asarray(input_embeddings, dtype=np.float32)
    xt = np.ascontiguousarray(X.T.astype(np.float16))

    wq_h = np.ascontiguousarray((np.asarray(Wq, np.float32) * 0.125).astype(np.float16))
    wk_h = np.ascontiguousarray(np.asarray(Wk, np.float32).astype(np.float16))
    wv_h = np.ascontiguousarray(np.asarray(Wv, np.float32).astype(np.float16))
    wo_h = np.ascontiguousarray(np.asarray(Wo, np.float32).astype(np.float16))
    bq_h = np.ascontiguousarray(np.asarray(bq, np.float32) * 0.125)
    bk_h = np.ascontiguousarray(np.asarray(bk, np.float32))
    bv_h = np.ascontiguousarray(np.asarray(bv, np.float32))

    in_maps = []
    for core in range(NCORES):
        r0 = core * SLAB
        d1 = np.ascontiguousarray(B16[r0 : r0 + SLAB, :].T)
        xtr = np.ascontiguousarray(xt[:, r0 : r0 + SLAB])
        in_maps.append(
            {
                "bt": bt,
                "d1": d1,
                "xt": xt,
                "xtr": xtr,
                "wq": wq_h,
                "wk": wk_h,
                "wv": wv_h,
                "wo": wo_h,
                "bq": bq_h,
                "bk": bk_h,
                "bv": bv_h,
            }
        )
    return in_maps


def kernel(**inputs) -> np.ndarray:
    if "nc" not in _NC_CACHE:
        _NC_CACHE["nc"] = build_bass()
    nc = _NC_CACHE["nc"]

    in_maps = _prep_host(**inputs)
    res = run_bass_kernel_spmd(nc, in_maps, core_ids=list(range(NCORES)))
    global LAST_RESULT
    LAST_RESULT = res
    bo = np.asarray(inputs["bo"], dtype=np.float32)
    slabs = [res.results[c]["out"] for c in range(NCORES)]
    return (np.concatenate(slabs, axis=0) + bo[None, :]).astype(np.float32)


if __name__ == "__main__":
    import reference

    inputs = {k: np.asarray(v) if not np.isscalar(v) else v for k, v in reference.setup_inputs().items()}
    got = kernel(**inputs)
    print("kernel output:", got.shape, got.dtype)


# revision 13
# speedup vs baseline: 1.3220x; 1.1124x over previous
"""Trainium2 Bass kernel for nn_AttentionLayer_78632261255284 (sparse_attention).

Strategy (8-way row sharding, fully transpose-free):
  Each core owns a slab of 512 query rows. The reachability-factor matrix
  slab is computed *transposed* ([4096 keys, 512 queries]) via the chain
  D_{k+1} = B^T @ D_k with lhsT = B tiles, which is exactly the layout the
  transposed attention scores need. Softmax uses the identity
  softmax(qk + log f) = f*exp(qk) / sum(f*exp(qk)) -- no log, no max
  subtraction (scores are bounded). The softmax denominator comes from an
  appended ones-column in V; the division is applied per head via a rank-1
  (ones x recip-row) PE broadcast. The output projection consumes the
  transposed per-head outputs directly as lhsT. Host adds bo at the end.

Numerics: fp16 operand storage everywhere (validated: L2 rel err ~5e-4),
fp32 PSUM accumulation. Factors are globally scaled by 2^-9 (cancels in
softmax normalization exactly); D3 is stored scaled by 1/4 to stay in fp16
range. All scale constants are powers of two (exact).
"""

import numpy as np

import concourse.bass as bass
import concourse.mybir as mybir
import concourse.tile as tile
from concourse import bacc
from concourse.bass_utils import run_bass_kernel_spmd

P = 128
N = 4096  # nodes (+virtual)
NB = N // P  # 32 node blocks
EMB = 512
ET = EMB // P  # 4 embed blocks
HEADS = 8
HD = 64
SLAB = 512  # rows per core
NCORES = 8
CHUNK = 32  # kb-blocks per attention chunk (full-head sexp, double-buffered)

dt = mybir.dt
AF = mybir.ActivationFunctionType
ALU = mybir.AluOpType

# factor-scale constants (powers of two; global 2^-9 scale cancels in softmax)
FSCALE = 1.0 / 512.0
C2 = 0.5 * FSCALE
C3 = 0.25 * FSCALE
C4 = 0.125 * FSCALE
D3_STORE = 0.25  # D3 stored as D3/4 (fp16 range); level-3 psum is D4/4

_NC_CACHE = {}
LAST_RESULT = None


def _install_ntff_shim():
    """Provide antenv.axon_hooks if the image lacks it, so trace=True under
    axon works (profiling via ctypes into libaxon_pjrt.so). No-op if the
    real module exists or the .so lacks the symbols."""
    try:
        from antenv.axon_hooks import get_axon_ntff_profile_hook  # noqa: F401
        return
    except ImportError:
        pass
    import contextlib
    import ctypes
    import sys
    import types

    so_path = "/opt/axon/libaxon_pjrt.so"
    hook = None
    try:
        lib = ctypes.CDLL(so_path)
        if hasattr(lib, "axon_start_nrt_profile"):
            lib.axon_start_nrt_profile.argtypes = [
                ctypes.POINTER(ctypes.c_int64),
                ctypes.c_size_t,
            ]
            lib.axon_start_nrt_profile.restype = ctypes.c_int64
            lib.axon_stop_nrt_profile.argtypes = [ctypes.c_char_p]
            lib.axon_stop_nrt_profile.restype = ctypes.c_int64

            @contextlib.contextmanager
            def _hook(output_dir, device_ids):
                import jax

                jax.devices()
                if device_ids:
                    ids = (ctypes.c_int64 * len(device_ids))(*device_ids)
                    rc = lib.axon_start_nrt_profile(ids, len(device_ids))
                else:
                    rc = lib.axon_start_nrt_profile(None, 0)
                if rc != 0:
                    raise RuntimeError(f"axon_start_nrt_profile rc={rc}")
                try:
                    yield
                finally:
                    n = lib.axon_stop_nrt_profile(str(output_dir).encode())
                    if n < 0:
                        raise RuntimeError(f"axon_stop_nrt_profile rc={n}")

            hook = _hook
    except OSError:
        pass

    mod = types.ModuleType("antenv.axon_hooks")
    mod.get_axon_ntff_profile_hook = lambda: hook
    mod.set_axon_ntff_profile_hook = lambda h: None
    sys.modules["antenv.axon_hooks"] = mod


_install_ntff_shim()


def build_bass():
    nc = bacc.Bacc("TRN2", target_bir_lowering=False, debug=False, num_devices=NCORES)

    bt = nc.dram_tensor("bt", [NB, P, NB, P], dt.float16, kind="ExternalInput")
    bt8 = nc.dram_tensor("bt8", [NB, P, NB, P], dt.float8e4, kind="ExternalInput")
    d18 = nc.dram_tensor("d18", [N, SLAB], dt.float8e4, kind="ExternalInput")
    d1 = nc.dram_tensor("d1", [N, SLAB], dt.float16, kind="ExternalInput")
    xt = nc.dram_tensor("xt", [EMB, N], dt.float16, kind="ExternalInput")
    xtr = nc.dram_tensor("xtr", [EMB, SLAB], dt.float16, kind="ExternalInput")
    wq = nc.dram_tensor("wq", [EMB, EMB], dt.float16, kind="ExternalInput")
    wk = nc.dram_tensor("wk", [EMB, EMB], dt.float16, kind="ExternalInput")
    wv = nc.dram_tensor("wv", [EMB, EMB], dt.float16, kind="ExternalInput")
    wo = nc.dram_tensor("wo", [EMB, EMB], dt.float16, kind="ExternalInput")
    bq = nc.dram_tensor("bq", [EMB], dt.float32, kind="ExternalInput")
    bk = nc.dram_tensor("bk", [EMB], dt.float32, kind="ExternalInput")
    bv = nc.dram_tensor("bv", [EMB], dt.float32, kind="ExternalInput")
    out = nc.dram_tensor("out", [SLAB, EMB], dt.float32, kind="ExternalOutput")

    with tile.TileContext(nc) as tc:
        with (
            tc.tile_pool(name="persist", bufs=1) as persist,
            tc.tile_pool(name="psA", bufs=3, space="PSUM") as psA,
            tc.tile_pool(name="psB", bufs=2, space="PSUM") as psB,
            tc.tile_pool(name="psR", bufs=1, space="PSUM") as psR,
        ):
            # ---------------- persistent tiles ----------------
            F = persist.tile([P, NB, SLAB], dt.float16, tag="F")
            qT = persist.tile([P, ET, SLAB], dt.float16, tag="qT")
            out_allT = persist.tile([P, ET, SLAB], dt.float16, tag="out_allT")
            wo_sb = persist.tile([P, ET, EMB], dt.float16, tag="wo_sb")
            bv_sb = persist.tile([P, ET], dt.float32, tag="bv_sb")
            ones64 = persist.tile([1, HD], dt.float16, tag="ones64")

            nc.sync.dma_start(wo_sb[:], wo.rearrange("(t p) c -> p t c", p=P))
            nc.sync.dma_start(bv_sb[:], bv.rearrange("(t p) -> p t", p=P))
            nc.vector.memset(ones64[:], 1.0)

            # ---------------- phase M: mask chain ----------------
            with tc.tile_pool(name="dchain", bufs=1) as dchain, tc.tile_pool(
                name="btile", bufs=3
            ) as btile:
                D_a8 = dchain.tile([P, NB, SLAB], dt.float8e4, tag="D_a8")
                D_b = dchain.tile([P, NB, SLAB], dt.float16, tag="D_b")
                D_c = dchain.tile([P, NB, SLAB], dt.float16, tag="D_c")
                d18r = d18.rearrange("(kb p) q -> p kb q", p=P)
                for kb in range(NB):
                    nc.sync.dma_start(D_a8[:, kb, :], d18r[:, kb, :])

                # level 1: fp8e4m3 + DoubleRow (B and D1 entries are exact 0/1)
                for m in range(NB):
                    bm8 = btile.tile([P, NB, P], dt.float8e4, tag="bm8")
                    nc.sync.dma_start(bm8[:], bt8[m])
                    ps = psA.tile([P, SLAB], dt.float32, tag="acc")
                    for k2 in range(NB // 2):
                        nc.tensor.matmul(
                            ps[:],
                            bm8[:, 2 * k2 : 2 * k2 + 2, :],
                            D_a8[:, 2 * k2 : 2 * k2 + 2, :],
                            start=(k2 == 0),
                            stop=(k2 == NB // 2 - 1),
                            perf_mode=mybir.MatmulPerfMode.DoubleRow,
                        )
                    nc.vector.tensor_scalar_mul(F[:, m, :], ps[:], C2)
                    nc.scalar.copy(D_b[:, m, :], ps[:])

                # levels 2, 3: fp16
                for level in (1, 2):
                    src = D_b if level == 1 else D_c
                    dst = D_c if level == 1 else None
                    for m in range(NB):
                        bm = btile.tile([P, NB, P], dt.float16, tag="bm")
                        nc.sync.dma_start(bm[:], bt[m])
                        ps = psA.tile([P, SLAB], dt.float32, tag="acc")
                        for kb in range(NB):
                            nc.tensor.matmul(
                                ps[:],
                                bm[:, kb, :],
                                src[:, kb, :],
                                start=(kb == 0),
                                stop=(kb == NB - 1),
                            )
                        if level == 1:
                            nc.vector.scalar_tensor_tensor(
                                out=F[:, m, :], in0=ps[:], scalar=C3, in1=F[:, m, :],
                                op0=ALU.mult, op1=ALU.max,
                            )
                            nc.scalar.mul(dst[:, m, :], ps[:], D3_STORE)
                        else:
                            nc.vector.scalar_tensor_tensor(
                                out=F[:, m, :], in0=ps[:], scalar=C4 * 4.0, in1=F[:, m, :],
                                op0=ALU.mult, op1=ALU.max,
                            )

            # ---------------- phase P: projections ----------------
            with tc.tile_pool(name="kv", bufs=1) as kv:
                kT = kv.tile([P, ET, N], dt.float16, tag="kT")
                v_sb = kv.tile([P, NB, HEADS, HD + 1], dt.float16, tag="v_sb")

                with tc.tile_pool(name="proj", bufs=1) as proj:
                    xt_sb = proj.tile([P, ET, N], dt.float16, tag="xt_sb")
                    xtr_sb = proj.tile([P, ET, SLAB], dt.float16, tag="xtr_sb")
                    wq_sb = proj.tile([P, ET, EMB], dt.float16, tag="wq_sb")
                    wk_sb = proj.tile([P, ET, EMB], dt.float16, tag="wk_sb")
                    wv_sb = proj.tile([P, ET, EMB], dt.float16, tag="wv_sb")
                    bq_sb = proj.tile([P, ET], dt.float32, tag="bq_sb")
                    bk_sb = proj.tile([P, ET], dt.float32, tag="bk_sb")

                    nc.sync.dma_start(xt_sb[:], xt.rearrange("(t p) n -> p t n", p=P))
                    nc.sync.dma_start(xtr_sb[:], xtr.rearrange("(t p) q -> p t q", p=P))
                    nc.sync.dma_start(wq_sb[:], wq.rearrange("(t p) c -> p t c", p=P))
                    nc.sync.dma_start(wk_sb[:], wk.rearrange("(t p) c -> p t c", p=P))
                    nc.sync.dma_start(wv_sb[:], wv.rearrange("(t p) c -> p t c", p=P))
                    nc.sync.dma_start(bq_sb[:], bq.rearrange("(t p) -> p t", p=P))
                    nc.sync.dma_start(bk_sb[:], bk.rearrange("(t p) -> p t", p=P))

                    # qT[hd, q] = (Wq' X_r^T) + bq'
                    for hb in range(ET):
                        ps = psA.tile([P, SLAB], dt.float32, tag="acc")
                        for t in range(ET):
                            nc.tensor.matmul(
                                ps[:],
                                wq_sb[:, t, hb * P : (hb + 1) * P],
                                xtr_sb[:, t, :],
                                start=(t == 0),
                                stop=(t == ET - 1),
                            )
                        nc.scalar.activation(
                            qT[:, hb, :], ps[:], AF.Identity, bias=bq_sb[:, hb : hb + 1]
                        )

                    # kT[hd, key] = (Wk X^T) + bk
                    for hb in range(ET):
                        for nck in range(N // SLAB):
                            ps = psA.tile([P, SLAB], dt.float32, tag="acc")
                            for t in range(ET):
                                nc.tensor.matmul(
                                    ps[:],
                                    wk_sb[:, t, hb * P : (hb + 1) * P],
                                    xt_sb[:, t, nck * SLAB : (nck + 1) * SLAB],
                                    start=(t == 0),
                                    stop=(t == ET - 1),
                                )
                            nc.scalar.activation(
                                kT[:, hb, nck * SLAB : (nck + 1) * SLAB],
                                ps[:],
                                AF.Identity,
                                bias=bk_sb[:, hb : hb + 1],
                            )

                    # V[node, hd] = X Wv   (bv added later per-partition on out'^T)
                    for nb in range(NB):
                        ps = psA.tile([P, SLAB], dt.float32, tag="acc")
                        for t in range(ET):
                            nc.tensor.matmul(
                                ps[:],
                                xt_sb[:, t, nb * P : (nb + 1) * P],
                                wv_sb[:, t, :],
                                start=(t == 0),
                                stop=(t == ET - 1),
                            )
                        nc.vector.tensor_copy(
                            v_sb[:, nb, :, 0:HD],
                            ps.rearrange("p (h d) -> p h d", h=HEADS),
                        )
                    nc.vector.memset(v_sb[:, :, :, HD : HD + 1], 1.0)

                # ---------------- phase A: attention ----------------
                with tc.tile_pool(name="attn", bufs=2) as attn, tc.tile_pool(
                    name="small", bufs=2
                ) as small:
                    for h in range(HEADS):
                        th = h // 2
                        po = (h % 2) * HD
                        po_tile = psB.tile([P, SLAB], dt.float32, tag="pout")
                        for ck in range(NB // CHUNK):
                            sexp = attn.tile([P, CHUNK, SLAB], dt.float16, tag="sexp")
                            for kc in range(CHUNK):
                                kb = ck * CHUNK + kc
                                ps = psA.tile([P, SLAB], dt.float32, tag="acc")
                                nc.tensor.matmul(
                                    ps[:],
                                    kT[po : po + HD, th, kb * P : (kb + 1) * P],
                                    qT[po : po + HD, th, :],
                                    start=True,
                                    stop=True,
                                )
                                nc.scalar.activation(sexp[:, kc, :], ps[:], AF.Exp)
                                nc.vector.tensor_tensor(
                                    out=sexp[:, kc, :], in0=sexp[:, kc, :],
                                    in1=F[:, kb, :], op=ALU.mult,
                                )
                            for kc in range(CHUNK):
                                kb = ck * CHUNK + kc
                                nc.tensor.matmul(
                                    po_tile[0 : HD + 1, :],
                                    v_sb[:, kb, h, :],
                                    sexp[:, kc, :],
                                    start=(kb == 0),
                                    stop=(kb == NB - 1),
                                )

                        # softmax denominator: row HD holds sum(f*exp)
                        row = small.tile([1, SLAB], dt.float32, tag="row")
                        nc.vector.tensor_copy(row[:], po_tile[HD : HD + 1, :])
                        nc.vector.reciprocal(row[:], row[:])
                        row16 = small.tile([1, SLAB], dt.float16, tag="row16")
                        nc.vector.tensor_copy(row16[:], row[:])
                        rps = psR.tile([HD, SLAB], dt.float32, tag="rps")
                        nc.tensor.matmul(rps[:], ones64[:], row16[:], start=True, stop=True)
                        r_sb = small.tile([HD, SLAB], dt.float32, tag="r_sb")
                        nc.scalar.copy(r_sb[:], rps[:])

                        otmp = small.tile([HD, SLAB], dt.float32, tag="otmp")
                        nc.vector.tensor_tensor(
                            out=otmp[:], in0=po_tile[0:HD, :], in1=r_sb[:], op=ALU.mult
                        )
                        nc.vector.tensor_scalar_add(
                            out_allT[po : po + HD, th, :], otmp[:],
                            bv_sb[po : po + HD, th : th + 1],
                        )

                    # ---------------- phase O: output projection ----------------
                    for qb in range(ET):
                        ps = psA.tile([P, SLAB], dt.float32, tag="acc")
                        for t in range(ET):
                            nc.tensor.matmul(
                                ps[:],
                                out_allT[:, t, qb * P : (qb + 1) * P],
                                wo_sb[:, t, :],
                                start=(t == 0),
                                stop=(t == ET - 1),
                            )
                        fin = small.tile([P, SLAB], dt.float32, tag="fin")
                        nc.vector.tensor_copy(fin[:], ps[:])
                        nc.sync.dma_start(out[qb * P : (qb + 1) * P, :], fin[:])

    nc.compile()
    return nc


def _prep_host(input_embeddings, edge_index, num_nodes, Wq, bq, Wk, bk, Wv, bv, Wo, bo):
    n = int(num_nodes) + 1
    assert n == N

    B = np.zeros((n, n), dtype=np.float32)
    idx = np.arange(n)
    B[idx, idx] = 1.0
    e0 = np.asarray(edge_index[0], dtype=np.int64)
    e1 = np.asarray(edge_index[1], dtype=np.int64)
    B[e0, e1] = 1.0
    B[: n - 1, n - 1] = 1.0
    B[n - 1, : n - 1] = 1.0

    B16 = B.astype(np.float16)
    fp8 = mybir.dt.np(dt.float8e4)
    # bt[m, p, kb, f] = B[kb*128+p, m*128+f]
    bt = np.ascontiguousarray(
        B16.reshape(NB, P, NB, P).transpose(2, 1, 0, 3)
    )
    bt8 = bt.astype(fp8)

    X = np.asarray(input_embeddings, dtype=np.float32)
    xt = np.ascontiguousarray(X.T.astype(np.float16))

    wq_h = np.ascontiguousarray((np.asarray(Wq, np.float32) * 0.125).astype(np.float16))
    wk_h = np.ascontiguousarray(np.asarray(Wk, np.float32).astype(np.float16))
    wv_h = np.ascontiguousarray(np.asarray(Wv, np.float32).astype(np.float16))
    wo_h = np.ascontiguousarray(np.asarray(Wo, np.float32).astype(np.float16))
    bq_h = np.ascontiguousarray(np.asarray(bq, np.float32) * 0.125)
    bk_h = np.ascontiguousarray(np.asarray(bk, np.float32))
    bv_h = np.ascontiguousarray(np.asarray(bv, np.float32))

    in_maps = []
    for core in range(NCORES):
        r0 = core * SLAB
        d1 = np.ascontiguousarray(B16[r0 : r0 + SLAB, :].T)
        d18_a = d1.astype(fp8)
        xtr = np.ascontiguousarray(xt[:, r0 : r0 + SLAB])
        in_maps.append(
            {
                "bt": bt,
                "bt8": bt8,
                "d18": d18_a,
                "d1": d1,
                "xt": xt,
                "xtr": xtr,
                "wq": wq_h,
                "wk": wk_h,
                "wv": wv_h,
                "wo": wo_h,
                "bq": bq_h,
                "bk": bk_h,
                "bv": bv_h,
            }
        )
    return in_maps


def kernel(**inputs) -> np.ndarray:
    if "nc" not in _NC_CACHE:
        _NC_CACHE["nc"] = build_bass()
    nc = _NC_CACHE["nc"]

    in_maps = _prep_host(**inputs)
    res = run_bass_kernel_spmd(nc, in_maps, core_ids=list(range(NCORES)))
    global LAST_RESULT
    LAST_RESULT = res
    bo = np.asarray(inputs["bo"], dtype=np.float32)
    slabs = [res.results[c]["out"] for c in range(NCORES)]
    return (np.concatenate(slabs, axis=0) + bo[None, :]).astype(np.float32)


if __name__ == "__main__":
    import reference

    inputs = {k: np.asarray(v) if not np.isscalar(v) else v for k, v in reference.setup_inputs().items()}
    got = kernel(**inputs)
    print("kernel output:", got.shape, got.dtype)
